# revision 11
# baseline (speedup 1.0000x reference)
"""Trainium2 Bass kernel for a dense recurrent scan (nn_CXBPU_55611236549128).

Math (per timestep t, K=4 microsteps):
    inj  = x_t @ W_in.T + b_in                  scattered into sensory_indices
    h    = relu(h @ W_rec.T + scatter(inj))     microstep 0
    h    = relu(h @ W_rec.T)                    microsteps 1..K-1
    out_t = h[:, output_indices] @ W_out.T + b_out

Sharding: data-parallel over batch, 8 rows per core, W_rec replicated.

Per-core design (feature-major "hT" layout [128 partitions, 16 chunks x 8 batch]):
  - Single-pass fp16 matmuls, h-stationary: W_rec.T resident in SBUF and
    streamed as the moving operand (the fast streaming port), hT chunks as
    the 8-column stationary.  End-to-end error vs the fp32 reference is
    ~8e-4 (the recurrence is contractive, so per-step fp16 rounding damps).
  - 4 PE column groups (tile_position=(0,32j)) each stream their own
    k-tiles; rounds of 4 concurrent matmuls pipeline at the 512-col
    streaming cadence (~216 ns).
  - PSUM layout: 4 separate one-bank tiles for the 4 output-column banks
    plus 4 separate one-bank psumT tiles (one per k-group).  Separate
    tiles per bank are essential: a single multi-bank tile makes Tile's
    PSUM tracker serialize every bank's first matmul behind the previous
    bank's evacuation read (~1 us stall per bank).
  - Tail per bank n: two half-bank casts (DVE + ACT in parallel) fp32->fp16
    into batch-major evac, then 4 "transpose-sum" matmuls against a 0/1
    selector (i128) fold the 4 partition groups into feature-major psumT_n,
    then one DVE relu produces the hT chunk group.  Bank 3's transpose-sum
    + relu are deferred into the next microstep's instruction stream.
  - Injection is one extra tiny matmul per bank on microstep 0:
    lhsT = [x_t^T; 1] (8 partitions), rhs = scatter-expanded W_in/b_in.
  - Readout: 4 column-split accumulation chains over hT chunks, partials
    folded with one selector matmul, ACT-copied to an SBUF staging tile.
"""

import os
from contextlib import ExitStack

import numpy as np

N = 2048
B = 64
T = 128
NCORES = 8
BPC = B // NCORES  # 8 batch rows per core
NCHUNK = N // 128  # 16

_CACHE = {}


def _build_nc(n_steps):
    import concourse.bass as bass
    import concourse.mybir as mybir
    import concourse.tile as tile
    from concourse import bacc

    f32 = mybir.dt.float32
    f16 = mybir.dt.float16
    nc = bacc.Bacc(trn_type="TRN2")

    wt_d = nc.dram_tensor("wt", [N, N], f16, kind="ExternalInput")
    winj_d = nc.dram_tensor("winj", [8, N], f16, kind="ExternalInput")
    xt_d = nc.dram_tensor("xt", [8, n_steps * BPC], f16, kind="ExternalInput")
    wsel_d = nc.dram_tensor("wsel", [128, 2 * NCHUNK], f16, kind="ExternalInput")
    i128_d = nc.dram_tensor("i128", [128, BPC], f16, kind="ExternalInput")
    out_d = nc.dram_tensor("out", [2, n_steps * BPC], f32, kind="ExternalOutput")

    with tile.TileContext(nc) as tc, ExitStack() as ctx:
        const = ctx.enter_context(tc.tile_pool(name="const", bufs=1))
        hpool = ctx.enter_context(tc.tile_pool(name="h", bufs=3))
        epool = ctx.enter_context(tc.tile_pool(name="evac", bufs=3))
        rpool = ctx.enter_context(tc.tile_pool(name="prs", bufs=2))
        ppool = ctx.enter_context(tc.tile_pool(name="psum", bufs=1, space="PSUM"))

        # resident W^T slabs: slab kk (k-tile) at cols [kk*N, (kk+1)*N).
        wt = const.tile([128, NCHUNK * N], f16)
        for u in range(NCHUNK):
            eng = (nc.sync, nc.scalar, nc.gpsimd)[u % 3]
            eng.dma_start(wt[:, u * N : (u + 1) * N], wt_d[u * 128 : (u + 1) * 128, :])
        winj = const.tile([8, N], f16)
        nc.sync.dma_start(winj[:], winj_d[:])
        xt = const.tile([8, n_steps * BPC], f16)
        nc.scalar.dma_start(xt[:], xt_d[:])
        wsel = const.tile([128, 2 * NCHUNK], f16)
        nc.gpsimd.dma_start(wsel[:], wsel_d[:])
        i128 = const.tile([128, BPC], f16)
        nc.sync.dma_start(i128[:], i128_d[:])
        outst = const.tile([2, n_steps * BPC], f32)

        # PSUM: exactly 8 banks.  pbank[n] = output cols [512n, 512n+512);
        # psumT[n] = feature-major chunk group n (cols 0:32 used; the
        # remainder of banks 6/7 hosts the readout partials/sum).
        pbank = [ppool.tile([128, 512], f32, name=f"pbank{n}") for n in range(4)]
        psumT = [ppool.tile([128, 512], f32, name=f"psumT{n}") for n in range(4)]
        PR = 448  # col offset of readout scratch inside psumT[3]/psumT[2]

        # readout partial region read by a [128 x 8] DVE cast; zero the
        # never-written partitions once so the selector's 0-weights don't
        # multiply uninitialized PSUM.
        nc.vector.memset(psumT[3][:, PR : PR + BPC], 0.0)

        hT = hpool.tile([128, NCHUNK * BPC], f16)
        nc.vector.memset(hT[:], 0.0)

        tc.strict_bb_all_engine_barrier()

        # Each bank's tail (fine-grained casts -> per-chunk col-split
        # transpose-sum matmuls -> relu) is interleaved into the NEXT bank's
        # main rounds so every transpose matmul is data-ready when the PE
        # reaches it and stays within the PE's reorder window.  Bank 3's
        # tail (and the per-timestep readout) spill into the next
        # microstep's bank-0 emission via `pending`.
        pending = None  # (tmm_emitters[4], relu_emitter, extra_emitter|None)

        for t in range(n_steps):
            for s in range(4):
                evac = epool.tile([128, N], f16)
                hT_new = hpool.tile([128, NCHUNK * BPC], f16)

                def make_tail(n, evac=evac, hT_new=hT_new):
                    def tmm_pair(half, n=n, evac=evac):
                        # transpose-sum, col-group split ([128,32] stationary
                        # at strip q): psumT_n[32q+m, ci*8+b] =
                        #   sum_j evac[32j+b, (4n+ci)*128+32q+m]
                        for ci in (2 * half, 2 * half + 1):
                            c = 4 * n + ci
                            for q in range(4):
                                nc.tensor.matmul(
                                    psumT[n][32 * q : 32 * q + 32,
                                             ci * BPC : (ci + 1) * BPC],
                                    lhsT=evac[:, c * 128 + 32 * q :
                                              c * 128 + 32 * q + 32],
                                    rhs=i128[:],
                                    start=True,
                                    stop=True,
                                    tile_position=(0, 32 * q),
                                )

                    def relu_half(half, n=n, hT_new=hT_new):
                        o = 32 * n + 16 * half
                        nc.vector.tensor_relu(
                            hT_new[:, o : o + 16], psumT[n][:, 16 * half : 16 * half + 16]
                        )

                    return tmm_pair, relu_half

                def emit_bank(n, tail, extra, s=s, t=t, hT=hT, evac=evac):
                    # psum[32j+b, :] += sum_k h[b,k] Wrec[512n+c,k]; col group
                    # j handles k-tiles {4r+j}.  The previous bank's tail
                    # (tmm pair + relu half) is threaded between rounds, late
                    # enough that its casts have landed, and the relu halves
                    # precede round 3 (which consumes group 3's hT chunks
                    # when this is bank 0).
                    if s == 0:
                        # injection at the bank head (overlaps the previous
                        # bank's tail): lhsT = [x_t^T; 1; 0] (8 partitions),
                        # rhs = scatter-expanded [W_in; b_in] columns.
                        nc.tensor.matmul(
                            pbank[n][0:BPC, :],
                            lhsT=xt[:, t * BPC : (t + 1) * BPC],
                            rhs=winj[:, 512 * n : 512 * (n + 1)],
                            start=True,
                            stop=False,
                        )

                    def rounds(r):
                        for j in range(4):
                            kk = 4 * r + j
                            # start marks the first write to region j this
                            # microstep; on s==0 the injection matmul already
                            # opened region 0.
                            nc.tensor.matmul(
                                pbank[n][32 * j : 32 * j + BPC, :],
                                lhsT=hT[:, kk * BPC : (kk + 1) * BPC],
                                rhs=wt[:, kk * N + 512 * n : kk * N + 512 * (n + 1)],
                                start=(r == 0 and not (s == 0 and j == 0)),
                                stop=(r == 3),
                                tile_position=(0, 32 * j),
                            )

                    rounds(0)
                    rounds(1)
                    if tail:
                        tail[0](0)   # tmm chunks 0-1 (DVE cast half)
                        tail[1](0)   # relu half 0
                    rounds(2)
                    if tail:
                        tail[0](1)   # tmm chunks 2-3 (ACT cast half)
                        tail[1](1)   # relu half 1
                    rounds(3)
                    if extra:
                        extra()      # deferred readout chain
                    # casts: DVE takes chunks 0-1, ACT chunks 2-3 in parallel
                    nc.vector.tensor_copy(
                        evac[:, 512 * n : 512 * n + 256], pbank[n][:, 0:256]
                    )
                    nc.scalar.copy(
                        evac[:, 512 * n + 256 : 512 * n + 512], pbank[n][:, 256:512]
                    )

                emit_bank(0, pending[0] if pending else None,
                          pending[1] if pending else None)
                tail1 = make_tail(0)
                emit_bank(1, tail1, None)
                tail2 = make_tail(1)
                emit_bank(2, tail2, None)
                tail3 = make_tail(2)
                emit_bank(3, tail3, None)

                extra = None
                if s == 3:
                    def extra(t=t, hT_new=hT_new):
                        # 16-chunk accumulation chain (pipelines at the
                        # 8-col issue rate), then ACT-copied out.
                        for c in range(NCHUNK):
                            nc.tensor.matmul(
                                psumT[2][0:2, PR : PR + BPC],
                                lhsT=wsel[:, c * 2 : (c + 1) * 2],
                                rhs=hT_new[:, c * BPC : (c + 1) * BPC],
                                start=(c == 0),
                                stop=(c == NCHUNK - 1),
                            )
                        nc.scalar.copy(
                            outst[:, t * BPC : (t + 1) * BPC],
                            psumT[2][0:2, PR : PR + BPC],
                        )

                pending = (make_tail(3), extra)

                hT = hT_new

        tail, extra = pending
        tail[0](0)
        tail[1](0)
        tail[0](1)
        tail[1](1)
        if extra:
            extra()
        nc.sync.dma_start(out_d[:], outst[:])
    nc.compile()
    return nc


def _prep_inputs(inputs, W_rec, W_in, b_in, W_out, sensory_indices, output_indices,
                 n_steps):
    inputs = np.asarray(inputs, np.float32)
    W_rec = np.asarray(W_rec, np.float32)
    W_in = np.asarray(W_in, np.float32)
    b_in = np.asarray(b_in, np.float32)
    W_out = np.asarray(W_out, np.float32)
    sens = np.asarray(sensory_indices).astype(np.int64)
    oidx = np.asarray(output_indices).astype(np.int64)

    wt = np.ascontiguousarray(W_rec.T).astype(np.float16)

    # scatter-expanded readout weights, feature-major by chunk
    wsel_full = np.zeros((2, N), np.float32)
    np.add.at(wsel_full, (slice(None), oidx), W_out)
    wsel = np.ascontiguousarray(
        wsel_full.reshape(2, NCHUNK, 128).transpose(2, 1, 0).reshape(128, 2 * NCHUNK)
    ).astype(np.float16)

    i128 = (np.arange(128)[:, None] % 32 == np.arange(BPC)[None, :]).astype(np.float16)

    # scatter-expanded injection weights: rows 0-3 = W_in.T, row 4 = b_in
    winj_full = np.zeros((8, N), np.float32)
    np.add.at(winj_full, (slice(None, 4), sens), W_in.T)
    np.add.at(winj_full[4], sens, b_in)
    winj = winj_full.astype(np.float16)

    # per-core x_t stationary blocks: [8, T*8], rows 0-3 = x_t^T, row 4 = 1
    ncin = inputs.shape[2]
    xt_cores = []
    for g in range(NCORES):
        a = inputs[g * BPC : (g + 1) * BPC, :n_steps, :]  # [8, T, 4]
        x = np.zeros((8, n_steps, BPC), np.float32)
        x[:ncin] = a.transpose(2, 1, 0)
        x[4] = 1.0
        xt_cores.append(np.ascontiguousarray(x.reshape(8, n_steps * BPC)).astype(np.float16))

    return wt, winj, xt_cores, wsel, i128


def _run(inputs, W_rec, W_in, b_in, W_out, b_out, sensory_indices, output_indices,
         K, n_steps=T, trace=False):
    from concourse.bass_utils import run_bass_kernel_spmd

    assert int(K) == 4
    wt, winj, xt_cores, wsel, i128 = _prep_inputs(
        inputs, W_rec, W_in, b_in, W_out, sensory_indices, output_indices, n_steps)

    if n_steps not in _CACHE:
        _CACHE[n_steps] = _build_nc(n_steps)
    nc = _CACHE[n_steps]

    in_maps = [
        {"wt": wt, "winj": winj, "xt": xt_cores[g], "wsel": wsel, "i128": i128}
        for g in range(NCORES)
    ]
    res = run_bass_kernel_spmd(nc, in_maps, list(range(NCORES)), trace=trace)

    b_out = np.asarray(b_out, np.float32)
    outs = []
    for g in range(NCORES):
        r = np.asarray(res.results[g]["out"])  # [2, T*8]
        outs.append(r.reshape(2, n_steps, BPC).transpose(2, 1, 0))  # [8, T, 2]
    full = np.concatenate(outs, axis=0) + b_out  # [B, T, 2]
    return np.ascontiguousarray(full.astype(np.float32)), res


def kernel(**inputs):
    out, _ = _run(
        inputs["inputs"], inputs["W_rec"], inputs["W_in"], inputs["b_in"],
        inputs["W_out"], inputs["b_out"], inputs["sensory_indices"],
        inputs["output_indices"], inputs["K"],
    )
    return out


# revision 13
# speedup vs baseline: 1.3154x; 1.3154x over previous
"""Trainium2 Bass kernel for a dense recurrent scan (nn_CXBPU_55611236549128).

Math (per timestep t, K=4 microsteps):
    inj  = x_t @ W_in.T + b_in                  scattered into sensory_indices
    h    = relu(h @ W_rec.T + scatter(inj))     microstep 0
    h    = relu(h @ W_rec.T)                    microsteps 1..K-1
    out_t = h[:, output_indices] @ W_out.T + b_out

Sharding: data-parallel over batch, 8 rows per core, W_rec replicated.

Per-core design (feature-major "hT" layout [128 partitions, 16 chunks x 8 batch]):
  - Single-pass fp16 matmuls, h-stationary: W_rec.T resident in SBUF and
    streamed as the moving operand (the fast streaming port), hT chunks as
    the 8-column stationary.  End-to-end error vs the fp32 reference is
    ~8e-4 (the recurrence is contractive, so per-step fp16 rounding damps).
  - 4 PE column groups (tile_position=(0,32j)) each stream their own
    k-tiles; rounds of 4 concurrent matmuls pipeline at the 512-col
    streaming cadence (~216 ns).
  - PSUM layout: 4 separate one-bank tiles for the 4 output-column banks
    plus 4 separate one-bank psumT tiles (one per k-group).  Separate
    tiles per bank are essential: a single multi-bank tile makes Tile's
    PSUM tracker serialize every bank's first matmul behind the previous
    bank's evacuation read (~1 us stall per bank).
  - Tail per bank n: two half-bank casts (DVE + ACT in parallel) fp32->fp16
    into batch-major evac, then 4 "transpose-sum" matmuls against a 0/1
    selector (i128) fold the 4 partition groups into feature-major psumT_n,
    then one DVE relu produces the hT chunk group.  Bank 3's transpose-sum
    + relu are deferred into the next microstep's instruction stream.
  - Injection is one extra tiny matmul per bank on microstep 0:
    lhsT = [x_t^T; 1] (8 partitions), rhs = scatter-expanded W_in/b_in.
  - Readout: 4 column-split accumulation chains over hT chunks, partials
    folded with one selector matmul, ACT-copied to an SBUF staging tile.
"""

import os
from contextlib import ExitStack

import numpy as np

N = 2048
B = 64
T = 128
NCORES = 8
BPC = B // NCORES  # 8 batch rows per core
NCHUNK = N // 128  # 16

_CACHE = {}


def _build_nc(n_steps):
    import concourse.bass as bass
    import concourse.mybir as mybir
    import concourse.tile as tile
    from concourse import bacc

    f32 = mybir.dt.float32
    f16 = mybir.dt.float16
    nc = bacc.Bacc(trn_type="TRN2")

    wt_d = nc.dram_tensor("wt", [N, N], f16, kind="ExternalInput")
    winj_d = nc.dram_tensor("winj", [8, N], f16, kind="ExternalInput")
    xt_d = nc.dram_tensor("xt", [8, n_steps * BPC], f16, kind="ExternalInput")
    wsel_d = nc.dram_tensor("wsel", [128, 2 * NCHUNK], f16, kind="ExternalInput")
    i128_d = nc.dram_tensor("i128", [128, BPC], f16, kind="ExternalInput")
    out_d = nc.dram_tensor("out", [2, n_steps * BPC], f32, kind="ExternalOutput")

    with tile.TileContext(nc) as tc, ExitStack() as ctx:
        const = ctx.enter_context(tc.tile_pool(name="const", bufs=1))
        hpool = ctx.enter_context(tc.tile_pool(name="h", bufs=3))
        epool = ctx.enter_context(tc.tile_pool(name="evac", bufs=3))
        rpool = ctx.enter_context(tc.tile_pool(name="prs", bufs=2))
        ppool = ctx.enter_context(tc.tile_pool(name="psum", bufs=1, space="PSUM"))

        # resident W^T slabs: slab kk (k-tile) at cols [kk*N, (kk+1)*N).
        wt = const.tile([128, NCHUNK * N], f16)
        for u in range(NCHUNK):
            eng = (nc.sync, nc.scalar, nc.gpsimd)[u % 3]
            eng.dma_start(wt[:, u * N : (u + 1) * N], wt_d[u * 128 : (u + 1) * 128, :])
        winj = const.tile([8, N], f16)
        nc.sync.dma_start(winj[:], winj_d[:])
        xt = const.tile([8, n_steps * BPC], f16)
        nc.scalar.dma_start(xt[:], xt_d[:])
        wsel = const.tile([128, 2 * NCHUNK], f16)
        nc.gpsimd.dma_start(wsel[:], wsel_d[:])
        i128 = const.tile([128, BPC], f16)
        nc.sync.dma_start(i128[:], i128_d[:])
        outst = const.tile([2, n_steps * BPC], f32)

        # PSUM: exactly 8 banks.  pbank[n] = output cols [512n, 512n+512);
        # psumT[n] = feature-major chunk group n (cols 0:32 used; the
        # remainder of banks 6/7 hosts the readout partials/sum).
        pbank = [ppool.tile([128, 512], f32, name=f"pbank{n}") for n in range(4)]
        psumT = [ppool.tile([128, 512], f32, name=f"psumT{n}") for n in range(4)]
        PR = 448  # col offset of readout scratch inside psumT[3]/psumT[2]

        # readout partial region read by a [128 x 8] DVE cast; zero the
        # never-written partitions once so the selector's 0-weights don't
        # multiply uninitialized PSUM.
        nc.vector.memset(psumT[3][:, PR : PR + BPC], 0.0)

        hT = hpool.tile([128, NCHUNK * BPC], f16)
        nc.vector.memset(hT[:], 0.0)

        tc.strict_bb_all_engine_barrier()

        # Each bank's tail (fine-grained casts -> per-chunk col-split
        # transpose-sum matmuls -> relu) is interleaved into the NEXT bank's
        # main rounds so every transpose matmul is data-ready when the PE
        # reaches it and stays within the PE's reorder window.  Bank 3's
        # tail (and the per-timestep readout) spill into the next
        # microstep's bank-0 emission via `pending`.
        pending = None  # (tmm_emitters[4], relu_emitter, extra_emitter|None)

        for t in range(n_steps):
            for s in range(4):
                evac = epool.tile([128, N], f16)
                hT_new = hpool.tile([128, NCHUNK * BPC], f16)

                def main_bank(n, s=s, t=t, hT=hT):
                    # psum[32j+b, :] += sum_k h[b,k] Wrec[512n+c,k]; col
                    # group j handles k-tiles {4r+j}.  On s==0 the injection
                    # matmul opens region 0 at the bank head, where it
                    # overlaps the previous bank's tail: lhsT = [x_t^T; 1; 0]
                    # (8 partitions), rhs = scatter-expanded [W_in; b_in].
                    if s == 0:
                        nc.tensor.matmul(
                            pbank[n][0:BPC, :],
                            lhsT=xt[:, t * BPC : (t + 1) * BPC],
                            rhs=winj[:, 512 * n : 512 * (n + 1)],
                            start=True,
                            stop=False,
                        )
                    for r in range(4):
                        for j in range(4):
                            kk = 4 * r + j
                            nc.tensor.matmul(
                                pbank[n][32 * j : 32 * j + BPC, :],
                                lhsT=hT[:, kk * BPC : (kk + 1) * BPC],
                                rhs=wt[:, kk * N + 512 * n : kk * N + 512 * (n + 1)],
                                start=(r == 0 and not (s == 0 and j == 0)),
                                stop=(r == 3),
                                tile_position=(0, 32 * j),
                            )

                def cast_bank(n, evac=evac):
                    # fp32 psum -> fp16 batch-major evac, halves on DVE and
                    # ACT in parallel to shorten the tail latency.
                    nc.vector.tensor_copy(
                        evac[:, 512 * n : 512 * n + 256], pbank[n][:, 0:256]
                    )
                    nc.scalar.copy(
                        evac[:, 512 * n + 256 : 512 * n + 512], pbank[n][:, 256:512]
                    )

                def tmm_group(n, evac=evac):
                    # transpose-sum: psumT_n[m, ci*8+b] = sum_j evac[32j+b, .]
                    for ci in range(4):
                        c = 4 * n + ci
                        nc.tensor.matmul(
                            psumT[n][:, ci * BPC : (ci + 1) * BPC],
                            lhsT=evac[:, c * 128 : (c + 1) * 128],
                            rhs=i128[:],
                            start=True,
                            stop=True,
                        )

                def relu_group(n, hT_new=hT_new):
                    nc.vector.tensor_relu(
                        hT_new[:, 32 * n : 32 * n + 32], psumT[n][:, 0:32]
                    )

                # flush deferred tail of the previous microstep first
                for fn in pending or []:
                    fn()

                main_bank(0)
                cast_bank(0)
                main_bank(1)
                cast_bank(1)
                tmm_group(0)
                relu_group(0)
                main_bank(2)
                cast_bank(2)
                tmm_group(1)
                relu_group(1)
                main_bank(3)
                cast_bank(3)
                tmm_group(2)
                relu_group(2)
                pending = [
                    lambda n=3, f=tmm_group: f(n),
                    lambda n=3, f=relu_group: f(n),
                ]

                if s == 3:
                    def readout(t=t, hT_new=hT_new):
                        # 16-chunk accumulation chain (pipelines at the
                        # 8-col issue rate), then ACT-copied out.
                        for c in range(NCHUNK):
                            nc.tensor.matmul(
                                psumT[2][0:2, PR : PR + BPC],
                                lhsT=wsel[:, c * 2 : (c + 1) * 2],
                                rhs=hT_new[:, c * BPC : (c + 1) * BPC],
                                start=(c == 0),
                                stop=(c == NCHUNK - 1),
                            )
                        nc.scalar.copy(
                            outst[:, t * BPC : (t + 1) * BPC],
                            psumT[2][0:2, PR : PR + BPC],
                        )

                    pending.append(readout)

                hT = hT_new

        for fn in pending:
            fn()
        nc.sync.dma_start(out_d[:], outst[:])
    nc.compile()
    return nc


def _prep_inputs(inputs, W_rec, W_in, b_in, W_out, sensory_indices, output_indices,
                 n_steps):
    inputs = np.asarray(inputs, np.float32)
    W_rec = np.asarray(W_rec, np.float32)
    W_in = np.asarray(W_in, np.float32)
    b_in = np.asarray(b_in, np.float32)
    W_out = np.asarray(W_out, np.float32)
    sens = np.asarray(sensory_indices).astype(np.int64)
    oidx = np.asarray(output_indices).astype(np.int64)

    wt = np.ascontiguousarray(W_rec.T).astype(np.float16)

    # scatter-expanded readout weights, feature-major by chunk
    wsel_full = np.zeros((2, N), np.float32)
    np.add.at(wsel_full, (slice(None), oidx), W_out)
    wsel = np.ascontiguousarray(
        wsel_full.reshape(2, NCHUNK, 128).transpose(2, 1, 0).reshape(128, 2 * NCHUNK)
    ).astype(np.float16)

    i128 = (np.arange(128)[:, None] % 32 == np.arange(BPC)[None, :]).astype(np.float16)

    # scatter-expanded injection weights: rows 0-3 = W_in.T, row 4 = b_in
    winj_full = np.zeros((8, N), np.float32)
    np.add.at(winj_full, (slice(None, 4), sens), W_in.T)
    np.add.at(winj_full[4], sens, b_in)
    winj = winj_full.astype(np.float16)

    # per-core x_t stationary blocks: [8, T*8], rows 0-3 = x_t^T, row 4 = 1
    ncin = inputs.shape[2]
    xt_cores = []
    for g in range(NCORES):
        a = inputs[g * BPC : (g + 1) * BPC, :n_steps, :]  # [8, T, 4]
        x = np.zeros((8, n_steps, BPC), np.float32)
        x[:ncin] = a.transpose(2, 1, 0)
        x[4] = 1.0
        xt_cores.append(np.ascontiguousarray(x.reshape(8, n_steps * BPC)).astype(np.float16))

    return wt, winj, xt_cores, wsel, i128


def _run(inputs, W_rec, W_in, b_in, W_out, b_out, sensory_indices, output_indices,
         K, n_steps=T, trace=False):
    from concourse.bass_utils import run_bass_kernel_spmd

    assert int(K) == 4
    wt, winj, xt_cores, wsel, i128 = _prep_inputs(
        inputs, W_rec, W_in, b_in, W_out, sensory_indices, output_indices, n_steps)

    if n_steps not in _CACHE:
        _CACHE[n_steps] = _build_nc(n_steps)
    nc = _CACHE[n_steps]

    in_maps = [
        {"wt": wt, "winj": winj, "xt": xt_cores[g], "wsel": wsel, "i128": i128}
        for g in range(NCORES)
    ]
    res = run_bass_kernel_spmd(nc, in_maps, list(range(NCORES)), trace=trace)

    b_out = np.asarray(b_out, np.float32)
    outs = []
    for g in range(NCORES):
        r = np.asarray(res.results[g]["out"])  # [2, T*8]
        outs.append(r.reshape(2, n_steps, BPC).transpose(2, 1, 0))  # [8, T, 2]
    full = np.concatenate(outs, axis=0) + b_out  # [B, T, 2]
    return np.ascontiguousarray(full.astype(np.float32)), res


def kernel(**inputs):
    out, _ = _run(
        inputs["inputs"], inputs["W_rec"], inputs["W_in"], inputs["b_in"],
        inputs["W_out"], inputs["b_out"], inputs["sensory_indices"],
        inputs["output_indices"], inputs["K"],
    )
    return out


# revision 15
# speedup vs baseline: 1.3276x; 1.0093x over previous
"""Trainium2 Bass kernel for a dense recurrent scan (nn_CXBPU_55611236549128).

Math (per timestep t, K=4 microsteps):
    inj  = x_t @ W_in.T + b_in                  scattered into sensory_indices
    h    = relu(h @ W_rec.T + scatter(inj))     microstep 0
    h    = relu(h @ W_rec.T)                    microsteps 1..K-1
    out_t = h[:, output_indices] @ W_out.T + b_out

Sharding: data-parallel over batch, 8 rows per core, W_rec replicated.

Per-core design (feature-major "hT" layout [128 partitions, 16 chunks x 8 batch]):
  - Single-pass fp16 matmuls, h-stationary: W_rec.T resident in SBUF and
    streamed as the moving operand (the fast streaming port), hT chunks as
    the 8-column stationary.  End-to-end error vs the fp32 reference is
    ~8e-4 (the recurrence is contractive, so per-step fp16 rounding damps).
  - 4 PE column groups (tile_position=(0,32j)) each stream their own
    k-tiles; rounds of 4 concurrent matmuls pipeline at the 512-col
    streaming cadence (~216 ns).
  - PSUM layout: 4 separate one-bank tiles for the 4 output-column banks
    plus 4 separate one-bank psumT tiles (one per k-group).  Separate
    tiles per bank are essential: a single multi-bank tile makes Tile's
    PSUM tracker serialize every bank's first matmul behind the previous
    bank's evacuation read (~1 us stall per bank).
  - Tail per bank n: two half-bank casts (DVE + ACT in parallel) fp32->fp16
    into batch-major evac, then 4 "transpose-sum" matmuls against a 0/1
    selector (i128) fold the 4 partition groups into feature-major psumT_n,
    then one DVE relu produces the hT chunk group.  Bank 3's transpose-sum
    + relu are deferred into the next microstep's instruction stream.
  - Injection is one extra tiny matmul per bank on microstep 0:
    lhsT = [x_t^T; 1] (8 partitions), rhs = scatter-expanded W_in/b_in.
  - Readout: 4 column-split accumulation chains over hT chunks, partials
    folded with one selector matmul, ACT-copied to an SBUF staging tile.
"""

import os
from contextlib import ExitStack

import numpy as np

N = 2048
B = 64
T = 128
NCORES = 8
BPC = B // NCORES  # 8 batch rows per core
NCHUNK = N // 128  # 16

_CACHE = {}


def _build_nc(n_steps):
    import concourse.bass as bass
    import concourse.mybir as mybir
    import concourse.tile as tile
    from concourse import bacc

    f32 = mybir.dt.float32
    f16 = mybir.dt.float16
    nc = bacc.Bacc(trn_type="TRN2")

    wt_d = nc.dram_tensor("wt", [N, N], f16, kind="ExternalInput")
    winj_d = nc.dram_tensor("winj", [8, N], f16, kind="ExternalInput")
    xt_d = nc.dram_tensor("xt", [8, n_steps * BPC], f16, kind="ExternalInput")
    wsel_d = nc.dram_tensor("wsel", [128, 2 * NCHUNK], f16, kind="ExternalInput")
    i128_d = nc.dram_tensor("i128", [128, BPC], f16, kind="ExternalInput")
    out_d = nc.dram_tensor("out", [2, n_steps * BPC], f32, kind="ExternalOutput")

    with tile.TileContext(nc) as tc, ExitStack() as ctx:
        const = ctx.enter_context(tc.tile_pool(name="const", bufs=1))
        hpool = ctx.enter_context(tc.tile_pool(name="h", bufs=3))
        epool = ctx.enter_context(tc.tile_pool(name="evac", bufs=3))
        rpool = ctx.enter_context(tc.tile_pool(name="prs", bufs=2))
        ppool = ctx.enter_context(tc.tile_pool(name="psum", bufs=1, space="PSUM"))

        # resident W^T slabs: slab kk (k-tile) at cols [kk*N, (kk+1)*N).
        wt = const.tile([128, NCHUNK * N], f16)
        for u in range(NCHUNK):
            eng = (nc.sync, nc.scalar, nc.gpsimd)[u % 3]
            eng.dma_start(wt[:, u * N : (u + 1) * N], wt_d[u * 128 : (u + 1) * 128, :])
        winj = const.tile([8, N], f16)
        nc.sync.dma_start(winj[:], winj_d[:])
        xt = const.tile([8, n_steps * BPC], f16)
        nc.scalar.dma_start(xt[:], xt_d[:])
        wsel = const.tile([128, 2 * NCHUNK], f16)
        nc.gpsimd.dma_start(wsel[:], wsel_d[:])
        i128 = const.tile([128, BPC], f16)
        nc.sync.dma_start(i128[:], i128_d[:])
        outst = const.tile([2, n_steps * BPC], f32)

        # PSUM: exactly 8 banks.  pbank[n] = output cols [512n, 512n+512);
        # psumT[n] = feature-major chunk group n (cols 0:32 used; the
        # remainder of banks 6/7 hosts the readout partials/sum).
        pbank = [ppool.tile([128, 512], f32, name=f"pbank{n}") for n in range(4)]
        psumT = [ppool.tile([128, 512], f32, name=f"psumT{n}") for n in range(4)]
        PR = 448  # col offset of readout scratch inside psumT[3]/psumT[2]

        # readout partial region read by a [128 x 8] DVE cast; zero the
        # never-written partitions once so the selector's 0-weights don't
        # multiply uninitialized PSUM.
        nc.vector.memset(psumT[3][:, PR : PR + BPC], 0.0)

        hT = hpool.tile([128, NCHUNK * BPC], f16)
        nc.vector.memset(hT[:], 0.0)

        tc.strict_bb_all_engine_barrier()

        # Each bank's tail (fine-grained casts -> per-chunk col-split
        # transpose-sum matmuls -> relu) is interleaved into the NEXT bank's
        # main rounds so every transpose matmul is data-ready when the PE
        # reaches it and stays within the PE's reorder window.  Bank 3's
        # tail (and the per-timestep readout) spill into the next
        # microstep's bank-0 emission via `pending`.
        pending = None  # (tmm_emitters[4], relu_emitter, extra_emitter|None)

        for t in range(n_steps):
            for s in range(4):
                evac = epool.tile([128, N], f16)
                hT_new = hpool.tile([128, NCHUNK * BPC], f16)

                def inj_mm(n, s=s, t=t):
                    # On s==0 the injection matmul opens region 0 at the bank
                    # head, where it overlaps the previous bank's tail:
                    # lhsT = [x_t^T; 1; 0] (8 partitions), rhs =
                    # scatter-expanded [W_in; b_in].
                    nc.tensor.matmul(
                        pbank[n][0:BPC, :],
                        lhsT=xt[:, t * BPC : (t + 1) * BPC],
                        rhs=winj[:, 512 * n : 512 * (n + 1)],
                        start=True,
                        stop=False,
                    )

                def rounds(n, rs, s=s, hT=hT):
                    # psum[32j+b, :] += sum_k h[b,k] Wrec[512n+c,k]; col
                    # group j handles k-tiles {4r+j}.
                    for r in rs:
                        for j in range(4):
                            kk = 4 * r + j
                            nc.tensor.matmul(
                                pbank[n][32 * j : 32 * j + BPC, :],
                                lhsT=hT[:, kk * BPC : (kk + 1) * BPC],
                                rhs=wt[:, kk * N + 512 * n : kk * N + 512 * (n + 1)],
                                start=(r == 0 and not (s == 0 and j == 0)),
                                stop=(r == 3),
                                tile_position=(0, 32 * j),
                            )

                def main_bank(n, s=s):
                    if s == 0:
                        inj_mm(n)
                    rounds(n, range(4))

                def cast_bank(n, evac=evac):
                    # fp32 psum -> fp16 batch-major evac, halves on DVE and
                    # ACT in parallel to shorten the tail latency.
                    nc.vector.tensor_copy(
                        evac[:, 512 * n : 512 * n + 256], pbank[n][:, 0:256]
                    )
                    nc.scalar.copy(
                        evac[:, 512 * n + 256 : 512 * n + 512], pbank[n][:, 256:512]
                    )

                def tmm_group(n, evac=evac):
                    # transpose-sum: psumT_n[m, ci*8+b] = sum_j evac[32j+b, .]
                    for ci in range(4):
                        c = 4 * n + ci
                        nc.tensor.matmul(
                            psumT[n][:, ci * BPC : (ci + 1) * BPC],
                            lhsT=evac[:, c * 128 : (c + 1) * 128],
                            rhs=i128[:],
                            start=True,
                            stop=True,
                        )

                def relu_group(n, hT_new=hT_new):
                    nc.vector.tensor_relu(
                        hT_new[:, 32 * n : 32 * n + 32], psumT[n][:, 0:32]
                    )

                # Bank 0's rounds 0-2 depend only on relu groups 0-2 of the
                # previous microstep, so they stream DURING the deferred
                # bank-3 tail's latency window; round 3 (which consumes
                # group 3) and the readout chain follow the flush.
                if s == 0:
                    inj_mm(0)
                rounds(0, range(3))
                for fn in (pending[:2] if pending else []):
                    fn()   # tmm_group(3)', relu_group(3)'
                rounds(0, [3])
                for fn in (pending[2:] if pending else []):
                    fn()   # readout'
                cast_bank(0)
                main_bank(1)
                cast_bank(1)
                tmm_group(0)
                relu_group(0)
                main_bank(2)
                cast_bank(2)
                tmm_group(1)
                relu_group(1)
                main_bank(3)
                cast_bank(3)
                tmm_group(2)
                relu_group(2)
                pending = [
                    lambda n=3, f=tmm_group: f(n),
                    lambda n=3, f=relu_group: f(n),
                ]

                if s == 3:
                    def readout(t=t, hT_new=hT_new):
                        # 16-chunk accumulation chain (pipelines at the
                        # 8-col issue rate), then ACT-copied out.
                        for c in range(NCHUNK):
                            nc.tensor.matmul(
                                psumT[2][0:2, PR : PR + BPC],
                                lhsT=wsel[:, c * 2 : (c + 1) * 2],
                                rhs=hT_new[:, c * BPC : (c + 1) * BPC],
                                start=(c == 0),
                                stop=(c == NCHUNK - 1),
                            )
                        nc.scalar.copy(
                            outst[:, t * BPC : (t + 1) * BPC],
                            psumT[2][0:2, PR : PR + BPC],
                        )

                    pending.append(readout)

                hT = hT_new

        for fn in pending:
            fn()
        nc.sync.dma_start(out_d[:], outst[:])
    nc.compile()
    return nc


def _prep_inputs(inputs, W_rec, W_in, b_in, W_out, sensory_indices, output_indices,
                 n_steps):
    inputs = np.asarray(inputs, np.float32)
    W_rec = np.asarray(W_rec, np.float32)
    W_in = np.asarray(W_in, np.float32)
    b_in = np.asarray(b_in, np.float32)
    W_out = np.asarray(W_out, np.float32)
    sens = np.asarray(sensory_indices).astype(np.int64)
    oidx = np.asarray(output_indices).astype(np.int64)

    wt = np.ascontiguousarray(W_rec.T).astype(np.float16)

    # scatter-expanded readout weights, feature-major by chunk
    wsel_full = np.zeros((2, N), np.float32)
    np.add.at(wsel_full, (slice(None), oidx), W_out)
    wsel = np.ascontiguousarray(
        wsel_full.reshape(2, NCHUNK, 128).transpose(2, 1, 0).reshape(128, 2 * NCHUNK)
    ).astype(np.float16)

    i128 = (np.arange(128)[:, None] % 32 == np.arange(BPC)[None, :]).astype(np.float16)

    # scatter-expanded injection weights: rows 0-3 = W_in.T, row 4 = b_in
    winj_full = np.zeros((8, N), np.float32)
    np.add.at(winj_full, (slice(None, 4), sens), W_in.T)
    np.add.at(winj_full[4], sens, b_in)
    winj = winj_full.astype(np.float16)

    # per-core x_t stationary blocks: [8, T*8], rows 0-3 = x_t^T, row 4 = 1
    ncin = inputs.shape[2]
    xt_cores = []
    for g in range(NCORES):
        a = inputs[g * BPC : (g + 1) * BPC, :n_steps, :]  # [8, T, 4]
        x = np.zeros((8, n_steps, BPC), np.float32)
        x[:ncin] = a.transpose(2, 1, 0)
        x[4] = 1.0
        xt_cores.append(np.ascontiguousarray(x.reshape(8, n_steps * BPC)).astype(np.float16))

    return wt, winj, xt_cores, wsel, i128


def _run(inputs, W_rec, W_in, b_in, W_out, b_out, sensory_indices, output_indices,
         K, n_steps=T, trace=False):
    from concourse.bass_utils import run_bass_kernel_spmd

    assert int(K) == 4
    wt, winj, xt_cores, wsel, i128 = _prep_inputs(
        inputs, W_rec, W_in, b_in, W_out, sensory_indices, output_indices, n_steps)

    if n_steps not in _CACHE:
        _CACHE[n_steps] = _build_nc(n_steps)
    nc = _CACHE[n_steps]

    in_maps = [
        {"wt": wt, "winj": winj, "xt": xt_cores[g], "wsel": wsel, "i128": i128}
        for g in range(NCORES)
    ]
    res = run_bass_kernel_spmd(nc, in_maps, list(range(NCORES)), trace=trace)

    b_out = np.asarray(b_out, np.float32)
    outs = []
    for g in range(NCORES):
        r = np.asarray(res.results[g]["out"])  # [2, T*8]
        outs.append(r.reshape(2, n_steps, BPC).transpose(2, 1, 0))  # [8, T, 2]
    full = np.concatenate(outs, axis=0) + b_out  # [B, T, 2]
    return np.ascontiguousarray(full.astype(np.float32)), res


def kernel(**inputs):
    out, _ = _run(
        inputs["inputs"], inputs["W_rec"], inputs["W_in"], inputs["b_in"],
        inputs["W_out"], inputs["b_out"], inputs["sensory_indices"],
        inputs["output_indices"], inputs["K"],
    )
    return out


# revision 16
# speedup vs baseline: 1.5056x; 1.1341x over previous
"""Trainium2 Bass kernel for a dense recurrent scan (nn_CXBPU_55611236549128).

Math (per timestep t, K=4 microsteps):
    inj  = x_t @ W_in.T + b_in                  scattered into sensory_indices
    h    = relu(h @ W_rec.T + scatter(inj))     microstep 0
    h    = relu(h @ W_rec.T)                    microsteps 1..K-1
    out_t = h[:, output_indices] @ W_out.T + b_out

Sharding: data-parallel over batch, 8 rows per core, W_rec replicated.

Per-core design (feature-major "hT" layout [128 partitions, 16 chunks x 8 batch]):
  - Single-pass fp16 matmuls, h-stationary: W_rec.T resident in SBUF and
    streamed as the moving operand (the fast streaming port), hT chunks as
    the 8-column stationary.  End-to-end error vs the fp32 reference is
    ~8e-4 (the recurrence is contractive, so per-step fp16 rounding damps).
  - 4 PE column groups (tile_position=(0,32j)) each stream their own
    k-tiles; rounds of 4 concurrent matmuls pipeline at the 512-col
    streaming cadence (~216 ns).
  - PSUM layout: 4 separate one-bank tiles for the 4 output-column banks
    plus 4 separate one-bank psumT tiles (one per k-group).  Separate
    tiles per bank are essential: a single multi-bank tile makes Tile's
    PSUM tracker serialize every bank's first matmul behind the previous
    bank's evacuation read (~1 us stall per bank).
  - Tail per bank n: two half-bank casts (DVE + ACT in parallel) fp32->fp16
    into batch-major evac, then 4 "transpose-sum" matmuls against a 0/1
    selector (i128) fold the 4 partition groups into feature-major psumT_n,
    then one DVE relu produces the hT chunk group.  Bank 3's transpose-sum
    + relu are deferred into the next microstep's instruction stream.
  - Injection is one extra tiny matmul per bank on microstep 0:
    lhsT = [x_t^T; 1] (8 partitions), rhs = scatter-expanded W_in/b_in.
  - Readout: 4 column-split accumulation chains over hT chunks, partials
    folded with one selector matmul, ACT-copied to an SBUF staging tile.
"""

import os
from contextlib import ExitStack

import numpy as np

N = 2048
B = 64
T = 128
NCORES = 8
BPC = B // NCORES  # 8 batch rows per core
NCHUNK = N // 128  # 16

_CACHE = {}


def _build_nc(n_steps):
    import concourse.bass as bass
    import concourse.mybir as mybir
    import concourse.tile as tile
    from concourse import bacc

    f32 = mybir.dt.float32
    f16 = mybir.dt.float16
    nc = bacc.Bacc(trn_type="TRN2")

    wt_d = nc.dram_tensor("wt", [N, N], f16, kind="ExternalInput")
    winj_d = nc.dram_tensor("winj", [8, N], f16, kind="ExternalInput")
    xt_d = nc.dram_tensor("xt", [8, n_steps * BPC], f16, kind="ExternalInput")
    wsel_d = nc.dram_tensor("wsel", [128, 2 * NCHUNK], f16, kind="ExternalInput")
    i128_d = nc.dram_tensor("i128", [128, BPC], f16, kind="ExternalInput")
    out_d = nc.dram_tensor("out", [2, n_steps * BPC], f32, kind="ExternalOutput")

    with tile.TileContext(nc) as tc, ExitStack() as ctx:
        const = ctx.enter_context(tc.tile_pool(name="const", bufs=1))
        hpool = ctx.enter_context(tc.tile_pool(name="h", bufs=3))
        epool = ctx.enter_context(tc.tile_pool(name="evac", bufs=3))
        rpool = ctx.enter_context(tc.tile_pool(name="prs", bufs=2))
        ppool = ctx.enter_context(tc.tile_pool(name="psum", bufs=1, space="PSUM"))

        # resident W^T slabs: slab kk (k-tile) at cols [kk*N, (kk+1)*N).
        wt = const.tile([128, NCHUNK * N], f16)
        for u in range(NCHUNK):
            eng = (nc.sync, nc.scalar, nc.gpsimd)[u % 3]
            eng.dma_start(wt[:, u * N : (u + 1) * N], wt_d[u * 128 : (u + 1) * 128, :])
        winj = const.tile([8, N], f16)
        nc.sync.dma_start(winj[:], winj_d[:])
        xt = const.tile([8, n_steps * BPC], f16)
        nc.scalar.dma_start(xt[:], xt_d[:])
        wsel = const.tile([128, 2 * NCHUNK], f16)
        nc.gpsimd.dma_start(wsel[:], wsel_d[:])
        i128 = const.tile([128, BPC], f16)
        nc.sync.dma_start(i128[:], i128_d[:])
        outst = const.tile([2, n_steps * BPC], f32)

        # PSUM: exactly 8 banks.  pbank[n] = output cols [512n, 512n+512);
        # psumT[n] = feature-major chunk group n (cols 0:32 used; the
        # remainder of banks 6/7 hosts the readout partials/sum).
        pbank = [ppool.tile([128, 512], f32, name=f"pbank{n}") for n in range(4)]
        psumT = [ppool.tile([128, 512], f32, name=f"psumT{n}") for n in range(4)]
        PR = 448  # col offset of readout scratch inside psumT[3]/psumT[2]

        # readout partial region read by a [128 x 8] DVE cast; zero the
        # never-written partitions once so the selector's 0-weights don't
        # multiply uninitialized PSUM.
        nc.vector.memset(psumT[3][:, PR : PR + BPC], 0.0)

        hT = hpool.tile([128, NCHUNK * BPC], f16)
        nc.vector.memset(hT[:], 0.0)

        tc.strict_bb_all_engine_barrier()

        # Each bank's tail (fine-grained casts -> per-chunk col-split
        # transpose-sum matmuls -> relu) is interleaved into the NEXT bank's
        # main rounds so every transpose matmul is data-ready when the PE
        # reaches it and stays within the PE's reorder window.  Bank 3's
        # tail (and the per-timestep readout) spill into the next
        # microstep's bank-0 emission via `pending`.
        pending = None  # (tmm_emitters[4], relu_emitter, extra_emitter|None)

        for t in range(n_steps):
            for s in range(4):
                evac = epool.tile([128, N], f16)
                hT_new = hpool.tile([128, NCHUNK * BPC], f16)

                def inj_mm(n, s=s, t=t):
                    # On s==0 the injection matmul opens region 0 at the bank
                    # head, where it overlaps the previous bank's tail:
                    # lhsT = [x_t^T; 1; 0] (8 partitions), rhs =
                    # scatter-expanded [W_in; b_in].
                    nc.tensor.matmul(
                        pbank[n][0:BPC, :],
                        lhsT=xt[:, t * BPC : (t + 1) * BPC],
                        rhs=winj[:, 512 * n : 512 * (n + 1)],
                        start=True,
                        stop=False,
                    )

                def rounds(n, rs, s=s, hT=hT):
                    # psum[32j+b, :] += sum_k h[b,k] Wrec[512n+c,k]; col
                    # group j handles k-tiles {4r+j}.
                    for r in rs:
                        for j in range(4):
                            kk = 4 * r + j
                            nc.tensor.matmul(
                                pbank[n][32 * j : 32 * j + BPC, :],
                                lhsT=hT[:, kk * BPC : (kk + 1) * BPC],
                                rhs=wt[:, kk * N + 512 * n : kk * N + 512 * (n + 1)],
                                start=(r == 0 and not (s == 0 and j == 0)),
                                stop=(r == 3),
                                tile_position=(0, 32 * j),
                            )

                def main_bank(n, s=s):
                    if s == 0:
                        inj_mm(n)
                    rounds(n, range(4))

                def cast_bank(n, evac=evac):
                    # fp32 psum -> fp16 batch-major evac, halves on DVE and
                    # ACT in parallel to shorten the tail latency.
                    nc.vector.tensor_copy(
                        evac[:, 512 * n : 512 * n + 256], pbank[n][:, 0:256]
                    )
                    nc.scalar.copy(
                        evac[:, 512 * n + 256 : 512 * n + 512], pbank[n][:, 256:512]
                    )

                def tmm_group(n, evac=evac):
                    # transpose-sum: psumT_n[m, ci*8+b] = sum_j evac[32j+b, .]
                    for ci in range(4):
                        c = 4 * n + ci
                        nc.tensor.matmul(
                            psumT[n][:, ci * BPC : (ci + 1) * BPC],
                            lhsT=evac[:, c * 128 : (c + 1) * 128],
                            rhs=i128[:],
                            start=True,
                            stop=True,
                        )

                def relu_group(n, hT_new=hT_new):
                    nc.vector.tensor_relu(
                        hT_new[:, 32 * n : 32 * n + 32], psumT[n][:, 0:32]
                    )

                # Emission is a topological schedule: MM starts are
                # pc-monotone on the PE, so every matmul is emitted at a
                # point where its inputs are already (or just-in-time)
                # available.  Rounds 0-2 of banks 0-2 only need relu groups
                # 0-2 of the previous microstep and stream through the
                # deferred bank-3 tail's latency window; the round-3 stops
                # follow the flushed tail; bank 3 and the in-microstep tails
                # fill the rest.
                if s == 0:
                    inj_mm(0)
                rounds(0, [0, 1, 2])
                if s == 0:
                    inj_mm(1)
                rounds(1, [0, 1, 2])
                for fn in (pending[:2] if pending else []):
                    fn()   # tmm_group(3)', relu_group(3)'
                if s == 0:
                    inj_mm(2)
                rounds(2, [0, 1, 2])
                rounds(0, [3])
                rounds(1, [3])
                rounds(2, [3])
                cast_bank(0)
                cast_bank(1)
                cast_bank(2)
                if s == 0:
                    inj_mm(3)
                rounds(3, [0, 1, 2])
                tmm_group(0)
                relu_group(0)
                for fn in (pending[2:] if pending else []):
                    fn()   # readout'
                rounds(3, [3])
                cast_bank(3)
                tmm_group(1)
                relu_group(1)
                tmm_group(2)
                relu_group(2)
                pending = [
                    lambda n=3, f=tmm_group: f(n),
                    lambda n=3, f=relu_group: f(n),
                ]

                if s == 3:
                    def readout(t=t, hT_new=hT_new):
                        # 16-chunk accumulation chain (pipelines at the
                        # 8-col issue rate), then ACT-copied out.
                        for c in range(NCHUNK):
                            nc.tensor.matmul(
                                psumT[2][0:2, PR : PR + BPC],
                                lhsT=wsel[:, c * 2 : (c + 1) * 2],
                                rhs=hT_new[:, c * BPC : (c + 1) * BPC],
                                start=(c == 0),
                                stop=(c == NCHUNK - 1),
                            )
                        nc.scalar.copy(
                            outst[:, t * BPC : (t + 1) * BPC],
                            psumT[2][0:2, PR : PR + BPC],
                        )

                    pending.append(readout)

                hT = hT_new

        for fn in pending:
            fn()
        nc.sync.dma_start(out_d[:], outst[:])
    nc.compile()
    return nc


def _prep_inputs(inputs, W_rec, W_in, b_in, W_out, sensory_indices, output_indices,
                 n_steps):
    inputs = np.asarray(inputs, np.float32)
    W_rec = np.asarray(W_rec, np.float32)
    W_in = np.asarray(W_in, np.float32)
    b_in = np.asarray(b_in, np.float32)
    W_out = np.asarray(W_out, np.float32)
    sens = np.asarray(sensory_indices).astype(np.int64)
    oidx = np.asarray(output_indices).astype(np.int64)

    wt = np.ascontiguousarray(W_rec.T).astype(np.float16)

    # scatter-expanded readout weights, feature-major by chunk
    wsel_full = np.zeros((2, N), np.float32)
    np.add.at(wsel_full, (slice(None), oidx), W_out)
    wsel = np.ascontiguousarray(
        wsel_full.reshape(2, NCHUNK, 128).transpose(2, 1, 0).reshape(128, 2 * NCHUNK)
    ).astype(np.float16)

    i128 = (np.arange(128)[:, None] % 32 == np.arange(BPC)[None, :]).astype(np.float16)

    # scatter-expanded injection weights: rows 0-3 = W_in.T, row 4 = b_in
    winj_full = np.zeros((8, N), np.float32)
    np.add.at(winj_full, (slice(None, 4), sens), W_in.T)
    np.add.at(winj_full[4], sens, b_in)
    winj = winj_full.astype(np.float16)

    # per-core x_t stationary blocks: [8, T*8], rows 0-3 = x_t^T, row 4 = 1
    ncin = inputs.shape[2]
    xt_cores = []
    for g in range(NCORES):
        a = inputs[g * BPC : (g + 1) * BPC, :n_steps, :]  # [8, T, 4]
        x = np.zeros((8, n_steps, BPC), np.float32)
        x[:ncin] = a.transpose(2, 1, 0)
        x[4] = 1.0
        xt_cores.append(np.ascontiguousarray(x.reshape(8, n_steps * BPC)).astype(np.float16))

    return wt, winj, xt_cores, wsel, i128


def _run(inputs, W_rec, W_in, b_in, W_out, b_out, sensory_indices, output_indices,
         K, n_steps=T, trace=False):
    from concourse.bass_utils import run_bass_kernel_spmd

    assert int(K) == 4
    wt, winj, xt_cores, wsel, i128 = _prep_inputs(
        inputs, W_rec, W_in, b_in, W_out, sensory_indices, output_indices, n_steps)

    if n_steps not in _CACHE:
        _CACHE[n_steps] = _build_nc(n_steps)
    nc = _CACHE[n_steps]

    in_maps = [
        {"wt": wt, "winj": winj, "xt": xt_cores[g], "wsel": wsel, "i128": i128}
        for g in range(NCORES)
    ]
    res = run_bass_kernel_spmd(nc, in_maps, list(range(NCORES)), trace=trace)

    b_out = np.asarray(b_out, np.float32)
    outs = []
    for g in range(NCORES):
        r = np.asarray(res.results[g]["out"])  # [2, T*8]
        outs.append(r.reshape(2, n_steps, BPC).transpose(2, 1, 0))  # [8, T, 2]
    full = np.concatenate(outs, axis=0) + b_out  # [B, T, 2]
    return np.ascontiguousarray(full.astype(np.float32)), res


def kernel(**inputs):
    out, _ = _run(
        inputs["inputs"], inputs["W_rec"], inputs["W_in"], inputs["b_in"],
        inputs["W_out"], inputs["b_out"], inputs["sensory_indices"],
        inputs["output_indices"], inputs["K"],
    )
    return out


# revision 21
# speedup vs baseline: 1.5065x; 1.0006x over previous
"""Trainium2 Bass kernel for a dense recurrent scan (nn_CXBPU_55611236549128).

Math (per timestep t, K=4 microsteps):
    inj  = x_t @ W_in.T + b_in                  scattered into sensory_indices
    h    = relu(h @ W_rec.T + scatter(inj))     microstep 0
    h    = relu(h @ W_rec.T)                    microsteps 1..K-1
    out_t = h[:, output_indices] @ W_out.T + b_out

Sharding: data-parallel over batch, 8 rows per core, W_rec replicated.

Per-core design (feature-major "hT" layout [128 partitions, 16 chunks x 8 batch]):
  - Single-pass fp16 matmuls, h-stationary: W_rec.T resident in SBUF and
    streamed as the moving operand (the fast streaming port), hT chunks as
    the 8-column stationary.  End-to-end error vs the fp32 reference is
    ~8e-4 (the recurrence is contractive, so per-step fp16 rounding damps).
  - 4 PE column groups (tile_position=(0,32j)) each stream their own
    k-tiles; rounds of 4 concurrent matmuls pipeline at the 512-col
    streaming cadence (~216 ns).
  - PSUM layout: 4 separate one-bank tiles for the 4 output-column banks
    plus 4 separate one-bank psumT tiles (one per k-group).  Separate
    tiles per bank are essential: a single multi-bank tile makes Tile's
    PSUM tracker serialize every bank's first matmul behind the previous
    bank's evacuation read (~1 us stall per bank).
  - Tail per bank n: two half-bank casts (DVE + ACT in parallel) fp32->fp16
    into batch-major evac, then 4 "transpose-sum" matmuls against a 0/1
    selector (i128) fold the 4 partition groups into feature-major psumT_n,
    then one DVE relu produces the hT chunk group.  Bank 3's transpose-sum
    + relu are deferred into the next microstep's instruction stream.
  - Injection is one extra tiny matmul per bank on microstep 0:
    lhsT = [x_t^T; 1] (8 partitions), rhs = scatter-expanded W_in/b_in.
  - Readout: a 16-chunk accumulation chain over hT against scatter-expanded
    W_out (pipelines at the 8-col issue rate) into a scratch corner of
    psumT[3]'s bank (whose PE writer is always a full microstep away),
    then ACT-copied to an SBUF staging tile, DMA'd out once at the end.
  - Emission order is a topological schedule: MM starts are pc-monotone on
    the PE, so rounds 0-2 of banks 0-2 (which need only relu groups 0-2)
    are emitted first and stream through the deferred bank-3 tail's
    latency window; round-3 stops, bank 3, and the tails follow.
    Measured ~97% PE occupancy, ~5.4 us per microstep (mains' streaming
    roofline is 3.46 us).
"""

import os
from contextlib import ExitStack

import numpy as np

N = 2048
B = 64
T = 128
NCORES = 8
BPC = B // NCORES  # 8 batch rows per core
NCHUNK = N // 128  # 16

_CACHE = {}


def _build_nc(n_steps):
    import concourse.bass as bass
    import concourse.mybir as mybir
    import concourse.tile as tile
    from concourse import bacc

    f32 = mybir.dt.float32
    f16 = mybir.dt.float16
    nc = bacc.Bacc(trn_type="TRN2")

    wt_d = nc.dram_tensor("wt", [N, N], f16, kind="ExternalInput")
    winj_d = nc.dram_tensor("winj", [8, N], f16, kind="ExternalInput")
    xt_d = nc.dram_tensor("xt", [8, n_steps * BPC], f16, kind="ExternalInput")
    wsel_d = nc.dram_tensor("wsel", [128, 2 * NCHUNK], f16, kind="ExternalInput")
    i128_d = nc.dram_tensor("i128", [128, BPC], f16, kind="ExternalInput")
    out_d = nc.dram_tensor("out", [2, n_steps * BPC], f32, kind="ExternalOutput")

    with tile.TileContext(nc) as tc, ExitStack() as ctx:
        const = ctx.enter_context(tc.tile_pool(name="const", bufs=1))
        hpool = ctx.enter_context(tc.tile_pool(name="h", bufs=3))
        epool = ctx.enter_context(tc.tile_pool(name="evac", bufs=3))
        rpool = ctx.enter_context(tc.tile_pool(name="prs", bufs=2))
        ppool = ctx.enter_context(tc.tile_pool(name="psum", bufs=1, space="PSUM"))

        # resident W^T slabs: slab kk (k-tile) at cols [kk*N, (kk+1)*N).
        wt = const.tile([128, NCHUNK * N], f16)
        for u in range(NCHUNK):
            eng = (nc.sync, nc.scalar, nc.gpsimd)[u % 3]
            eng.dma_start(wt[:, u * N : (u + 1) * N], wt_d[u * 128 : (u + 1) * 128, :])
        winj = const.tile([8, N], f16)
        nc.sync.dma_start(winj[:], winj_d[:])
        xt = const.tile([8, n_steps * BPC], f16)
        nc.scalar.dma_start(xt[:], xt_d[:])
        wsel = const.tile([128, 2 * NCHUNK], f16)
        nc.gpsimd.dma_start(wsel[:], wsel_d[:])
        i128 = const.tile([128, BPC], f16)
        nc.sync.dma_start(i128[:], i128_d[:])
        outst = const.tile([2, n_steps * BPC], f32)

        # PSUM: exactly 8 banks.  pbank[n] = output cols [512n, 512n+512);
        # psumT[n] = feature-major chunk group n (cols 0:32 used; a corner
        # of bank 7 hosts the readout accumulator).
        pbank = [ppool.tile([128, 512], f32, name=f"pbank{n}") for n in range(4)]
        psumT = [ppool.tile([128, 512], f32, name=f"psumT{n}") for n in range(4)]
        PR = 448  # col offset of readout scratch inside psumT[3]/psumT[2]

        # readout partial region read by a [128 x 8] DVE cast; zero the
        # never-written partitions once so the selector's 0-weights don't
        # multiply uninitialized PSUM.
        nc.vector.memset(psumT[3][:, PR : PR + BPC], 0.0)

        hT = hpool.tile([128, NCHUNK * BPC], f16)
        nc.vector.memset(hT[:], 0.0)

        tc.strict_bb_all_engine_barrier()

        # Bank 3's tail (and the per-timestep readout) spill into the next
        # microstep's emission via `pending`: [tmm_group(3), relu_group(3),
        # readout?].
        pending = None

        for t in range(n_steps):
            for s in range(4):
                evac = epool.tile([128, N], f16)
                hT_new = hpool.tile([128, NCHUNK * BPC], f16)

                def inj_mm(n, s=s, t=t):
                    # On s==0 the injection matmul opens region 0 at the bank
                    # head, where it overlaps the previous bank's tail:
                    # lhsT = [x_t^T; 1; 0] (8 partitions), rhs =
                    # scatter-expanded [W_in; b_in].
                    nc.tensor.matmul(
                        pbank[n][0:BPC, :],
                        lhsT=xt[:, t * BPC : (t + 1) * BPC],
                        rhs=winj[:, 512 * n : 512 * (n + 1)],
                        start=True,
                        stop=False,
                    )

                def rounds(n, rs, s=s, hT=hT):
                    # psum[32j+b, :] += sum_k h[b,k] Wrec[512n+c,k]; col
                    # group j handles k-tiles {4r+j}.
                    for r in rs:
                        for j in range(4):
                            kk = 4 * r + j
                            nc.tensor.matmul(
                                pbank[n][32 * j : 32 * j + BPC, :],
                                lhsT=hT[:, kk * BPC : (kk + 1) * BPC],
                                rhs=wt[:, kk * N + 512 * n : kk * N + 512 * (n + 1)],
                                start=(r == 0 and not (s == 0 and j == 0)),
                                stop=(r == 3),
                                tile_position=(0, 32 * j),
                            )

                def main_bank(n, s=s):
                    if s == 0:
                        inj_mm(n)
                    rounds(n, range(4))

                def cast_bank(n, evac=evac):
                    # fp32 psum -> fp16 batch-major evac, halves on DVE and
                    # ACT in parallel to shorten the tail latency.
                    nc.vector.tensor_copy(
                        evac[:, 512 * n : 512 * n + 256], pbank[n][:, 0:256]
                    )
                    nc.scalar.copy(
                        evac[:, 512 * n + 256 : 512 * n + 512], pbank[n][:, 256:512]
                    )

                def tmm_group(n, evac=evac):
                    # transpose-sum: psumT_n[m, ci*8+b] = sum_j evac[32j+b, .]
                    for ci in range(4):
                        c = 4 * n + ci
                        nc.tensor.matmul(
                            psumT[n][:, ci * BPC : (ci + 1) * BPC],
                            lhsT=evac[:, c * 128 : (c + 1) * 128],
                            rhs=i128[:],
                            start=True,
                            stop=True,
                        )

                def relu_group(n, hT_new=hT_new):
                    nc.vector.tensor_relu(
                        hT_new[:, 32 * n : 32 * n + 32], psumT[n][:, 0:32]
                    )

                # Emission is a topological schedule: MM starts are
                # pc-monotone on the PE, so every matmul is emitted at a
                # point where its inputs are already (or just-in-time)
                # available.  Rounds 0-2 of banks 0-2 only need relu groups
                # 0-2 of the previous microstep and stream through the
                # deferred bank-3 tail's latency window; the round-3 stops
                # follow the flushed tail; bank 3 and the in-microstep tails
                # fill the rest.
                if s == 0:
                    inj_mm(0)
                rounds(0, [0, 1, 2])
                if s == 0:
                    inj_mm(1)
                rounds(1, [0, 1, 2])
                for fn in (pending[:2] if pending else []):
                    fn()   # tmm_group(3)', relu_group(3)'
                if s == 0:
                    inj_mm(2)
                rounds(2, [0, 1, 2])
                rounds(0, [3])
                rounds(1, [3])
                rounds(2, [3])
                cast_bank(0)
                cast_bank(1)
                cast_bank(2)
                if s == 0:
                    inj_mm(3)
                rounds(3, [0, 1, 2])
                tmm_group(0)
                relu_group(0)
                for fn in (pending[2:] if pending else []):
                    fn()   # readout'
                rounds(3, [3])
                cast_bank(3)
                tmm_group(1)
                relu_group(1)
                tmm_group(2)
                relu_group(2)
                pending = [
                    lambda n=3, f=tmm_group: f(n),
                    lambda n=3, f=relu_group: f(n),
                ]

                if s == 3:
                    def readout(t=t, hT_new=hT_new):
                        # 16-chunk accumulation chain (pipelines at the
                        # 8-col issue rate), then ACT-copied out.
                        for c in range(NCHUNK):
                            nc.tensor.matmul(
                                psumT[3][0:2, PR : PR + BPC],
                                lhsT=wsel[:, c * 2 : (c + 1) * 2],
                                rhs=hT_new[:, c * BPC : (c + 1) * BPC],
                                start=(c == 0),
                                stop=(c == NCHUNK - 1),
                            )
                        nc.scalar.copy(
                            outst[:, t * BPC : (t + 1) * BPC],
                            psumT[3][0:2, PR : PR + BPC],
                        )

                    pending.append(readout)

                hT = hT_new

        for fn in pending:
            fn()
        nc.sync.dma_start(out_d[:], outst[:])
    nc.compile()
    return nc


def _prep_inputs(inputs, W_rec, W_in, b_in, W_out, sensory_indices, output_indices,
                 n_steps):
    inputs = np.asarray(inputs, np.float32)
    W_rec = np.asarray(W_rec, np.float32)
    W_in = np.asarray(W_in, np.float32)
    b_in = np.asarray(b_in, np.float32)
    W_out = np.asarray(W_out, np.float32)
    sens = np.asarray(sensory_indices).astype(np.int64)
    oidx = np.asarray(output_indices).astype(np.int64)

    wt = np.ascontiguousarray(W_rec.T).astype(np.float16)

    # scatter-expanded readout weights, feature-major by chunk
    wsel_full = np.zeros((2, N), np.float32)
    np.add.at(wsel_full, (slice(None), oidx), W_out)
    wsel = np.ascontiguousarray(
        wsel_full.reshape(2, NCHUNK, 128).transpose(2, 1, 0).reshape(128, 2 * NCHUNK)
    ).astype(np.float16)

    i128 = (np.arange(128)[:, None] % 32 == np.arange(BPC)[None, :]).astype(np.float16)

    # scatter-expanded injection weights: rows 0-3 = W_in.T, row 4 = b_in
    winj_full = np.zeros((8, N), np.float32)
    np.add.at(winj_full, (slice(None, 4), sens), W_in.T)
    np.add.at(winj_full[4], sens, b_in)
    winj = winj_full.astype(np.float16)

    # per-core x_t stationary blocks: [8, T*8], rows 0-3 = x_t^T, row 4 = 1
    ncin = inputs.shape[2]
    xt_cores = []
    for g in range(NCORES):
        a = inputs[g * BPC : (g + 1) * BPC, :n_steps, :]  # [8, T, 4]
        x = np.zeros((8, n_steps, BPC), np.float32)
        x[:ncin] = a.transpose(2, 1, 0)
        x[4] = 1.0
        xt_cores.append(np.ascontiguousarray(x.reshape(8, n_steps * BPC)).astype(np.float16))

    return wt, winj, xt_cores, wsel, i128


def _run(inputs, W_rec, W_in, b_in, W_out, b_out, sensory_indices, output_indices,
         K, n_steps=T, trace=False):
    from concourse.bass_utils import run_bass_kernel_spmd

    assert int(K) == 4
    wt, winj, xt_cores, wsel, i128 = _prep_inputs(
        inputs, W_rec, W_in, b_in, W_out, sensory_indices, output_indices, n_steps)

    if n_steps not in _CACHE:
        _CACHE[n_steps] = _build_nc(n_steps)
    nc = _CACHE[n_steps]

    in_maps = [
        {"wt": wt, "winj": winj, "xt": xt_cores[g], "wsel": wsel, "i128": i128}
        for g in range(NCORES)
    ]
    try:
        res = run_bass_kernel_spmd(nc, in_maps, list(range(NCORES)), trace=trace)
    except Exception:
        # transient device errors recover on re-execution
        res = run_bass_kernel_spmd(nc, in_maps, list(range(NCORES)), trace=trace)

    b_out = np.asarray(b_out, np.float32)
    outs = []
    for g in range(NCORES):
        r = np.asarray(res.results[g]["out"])  # [2, T*8]
        outs.append(r.reshape(2, n_steps, BPC).transpose(2, 1, 0))  # [8, T, 2]
    full = np.concatenate(outs, axis=0) + b_out  # [B, T, 2]
    return np.ascontiguousarray(full.astype(np.float32)), res


def kernel(**inputs):
    out, _ = _run(
        inputs["inputs"], inputs["W_rec"], inputs["W_in"], inputs["b_in"],
        inputs["W_out"], inputs["b_out"], inputs["sensory_indices"],
        inputs["output_indices"], inputs["K"],
    )
    return out


# revision 26
# speedup vs baseline: 1.5068x; 1.0002x over previous
"""Trainium2 Bass kernel for a dense recurrent scan (nn_CXBPU_55611236549128).

Math (per timestep t, K=4 microsteps):
    inj  = x_t @ W_in.T + b_in                  scattered into sensory_indices
    h    = relu(h @ W_rec.T + scatter(inj))     microstep 0
    h    = relu(h @ W_rec.T)                    microsteps 1..K-1
    out_t = h[:, output_indices] @ W_out.T + b_out

Sharding: data-parallel over batch, 8 rows per core, W_rec replicated.

Per-core design (feature-major "hT" layout [128 partitions, 16 chunks x 8 batch]):
  - Single-pass fp16 matmuls, h-stationary: W_rec.T resident in SBUF and
    streamed as the moving operand (the fast streaming port), hT chunks as
    the 8-column stationary.  End-to-end error vs the fp32 reference is
    ~8e-4 (the recurrence is contractive, so per-step fp16 rounding damps).
  - 4 PE column groups (tile_position=(0,32j)) each stream their own
    k-tiles; rounds of 4 concurrent matmuls pipeline at the 512-col
    streaming cadence (~216 ns).
  - PSUM layout: 4 separate one-bank tiles for the 4 output-column banks
    plus 4 separate one-bank psumT tiles (one per k-group).  Separate
    tiles per bank are essential: a single multi-bank tile makes Tile's
    PSUM tracker serialize every bank's first matmul behind the previous
    bank's evacuation read (~1 us stall per bank).
  - Tail per bank n: two half-bank casts (DVE + ACT in parallel) fp32->fp16
    into batch-major evac, then 4 "transpose-sum" matmuls against a 0/1
    selector (i128) fold the 4 partition groups into feature-major psumT_n,
    then one DVE relu produces the hT chunk group.  Bank 3's transpose-sum
    + relu are deferred into the next microstep's instruction stream.
  - Injection is one extra tiny matmul per bank on microstep 0:
    lhsT = [x_t^T; 1] (8 partitions), rhs = scatter-expanded W_in/b_in.
  - Readout: a 16-chunk accumulation chain over hT against scatter-expanded
    W_out (pipelines at the 8-col issue rate) into a scratch corner of
    psumT[3]'s bank (whose PE writer is always a full microstep away),
    then ACT-copied to an SBUF staging tile, DMA'd out once at the end.
  - Emission order is a topological schedule: MM starts are pc-monotone on
    the PE, so rounds 0-2 of banks 0-2 (which need only relu groups 0-2)
    are emitted first and stream through the deferred bank-3 tail's
    latency window; round-3 stops, bank 3, and the tails follow.
    Measured ~97% PE occupancy, ~5.4 us per microstep (mains' streaming
    roofline is 3.46 us).
"""

from contextlib import ExitStack

import numpy as np

N = 2048
B = 64
T = 128
NCORES = 8
BPC = B // NCORES  # 8 batch rows per core
NCHUNK = N // 128  # 16

_CACHE = {}


def _build_nc(n_steps):
    import concourse.bass as bass
    import concourse.mybir as mybir
    import concourse.tile as tile
    from concourse import bacc

    f32 = mybir.dt.float32
    f16 = mybir.dt.float16
    nc = bacc.Bacc(trn_type="TRN2")

    wt_d = nc.dram_tensor("wt", [N, N], f16, kind="ExternalInput")
    winj_d = nc.dram_tensor("winj", [8, N], f16, kind="ExternalInput")
    xt_d = nc.dram_tensor("xt", [8, n_steps * BPC], f16, kind="ExternalInput")
    wsel_d = nc.dram_tensor("wsel", [128, 2 * NCHUNK], f16, kind="ExternalInput")
    i128_d = nc.dram_tensor("i128", [128, BPC], f16, kind="ExternalInput")
    out_d = nc.dram_tensor("out", [2, n_steps * BPC], f32, kind="ExternalOutput")

    with tile.TileContext(nc) as tc, ExitStack() as ctx:
        const = ctx.enter_context(tc.tile_pool(name="const", bufs=1))
        hpool = ctx.enter_context(tc.tile_pool(name="h", bufs=3))
        epool = ctx.enter_context(tc.tile_pool(name="evac", bufs=3))
        rpool = ctx.enter_context(tc.tile_pool(name="prs", bufs=2))
        ppool = ctx.enter_context(tc.tile_pool(name="psum", bufs=1, space="PSUM"))

        # resident W^T slabs: slab kk (k-tile) at cols [kk*N, (kk+1)*N).
        wt = const.tile([128, NCHUNK * N], f16)
        for u in range(NCHUNK):
            eng = (nc.sync, nc.scalar, nc.gpsimd)[u % 3]
            eng.dma_start(wt[:, u * N : (u + 1) * N], wt_d[u * 128 : (u + 1) * 128, :])
        winj = const.tile([8, N], f16)
        nc.sync.dma_start(winj[:], winj_d[:])
        xt = const.tile([8, n_steps * BPC], f16)
        nc.scalar.dma_start(xt[:], xt_d[:])
        wsel = const.tile([128, 2 * NCHUNK], f16)
        nc.gpsimd.dma_start(wsel[:], wsel_d[:])
        i128 = const.tile([128, BPC], f16)
        nc.sync.dma_start(i128[:], i128_d[:])
        outst = const.tile([2, n_steps * BPC], f32)

        # PSUM: exactly 8 banks.  pbank[n] = output cols [512n, 512n+512);
        # psumT[n] = feature-major chunk group n (cols 0:32 used; a corner
        # of bank 7 hosts the readout accumulator).
        pbank = [ppool.tile([128, 512], f32, name=f"pbank{n}") for n in range(4)]
        psumT = [ppool.tile([128, 512], f32, name=f"psumT{n}") for n in range(4)]
        PR = 448  # col offset of readout scratch inside psumT[3]/psumT[2]

        # readout partial region read by a [128 x 8] DVE cast; zero the
        # never-written partitions once so the selector's 0-weights don't
        # multiply uninitialized PSUM.
        nc.vector.memset(psumT[3][:, PR : PR + BPC], 0.0)

        hT = hpool.tile([128, NCHUNK * BPC], f16)
        nc.vector.memset(hT[:], 0.0)

        tc.strict_bb_all_engine_barrier()

        # Bank 3's tail (and the per-timestep readout) spill into the next
        # microstep's emission via `pending`: [tmm_group(3), relu_group(3),
        # readout?].
        pending = None

        for t in range(n_steps):
            for s in range(4):
                evac = epool.tile([128, N], f16)
                hT_new = hpool.tile([128, NCHUNK * BPC], f16)

                def inj_mm(n, s=s, t=t):
                    # On s==0 the injection matmul opens region n at the bank
                    # head, where it overlaps the previous bank's tail:
                    # lhsT = [x_t^T; 1; 0] (8 partitions), rhs =
                    # scatter-expanded [W_in; b_in].  Bank n's injection
                    # rides column group n so no single group carries all
                    # four extra streams on s==0 microsteps.
                    nc.tensor.matmul(
                        pbank[n][32 * n : 32 * n + BPC, :],
                        lhsT=xt[:, t * BPC : (t + 1) * BPC],
                        rhs=winj[:, 512 * n : 512 * (n + 1)],
                        start=True,
                        stop=False,
                        tile_position=(0, 32 * n),
                    )

                def rounds(n, rs, s=s, hT=hT):
                    # psum[32j+b, :] += sum_k h[b,k] Wrec[512n+c,k]; col
                    # group j handles k-tiles {4r+j}.
                    for r in rs:
                        for j in range(4):
                            kk = 4 * r + j
                            nc.tensor.matmul(
                                pbank[n][32 * j : 32 * j + BPC, :],
                                lhsT=hT[:, kk * BPC : (kk + 1) * BPC],
                                rhs=wt[:, kk * N + 512 * n : kk * N + 512 * (n + 1)],
                                start=(r == 0 and not (s == 0 and j == n)),
                                stop=(r == 3),
                                tile_position=(0, 32 * j),
                            )

                def main_bank(n, s=s):
                    if s == 0:
                        inj_mm(n)
                    rounds(n, range(4))

                def cast_bank(n, evac=evac):
                    # fp32 psum -> fp16 batch-major evac, halves on DVE and
                    # ACT in parallel to shorten the tail latency.
                    nc.vector.tensor_copy(
                        evac[:, 512 * n : 512 * n + 256], pbank[n][:, 0:256]
                    )
                    nc.scalar.copy(
                        evac[:, 512 * n + 256 : 512 * n + 512], pbank[n][:, 256:512]
                    )

                def tmm_group(n, evac=evac):
                    # transpose-sum: psumT_n[m, ci*8+b] = sum_j evac[32j+b, .]
                    for ci in range(4):
                        c = 4 * n + ci
                        nc.tensor.matmul(
                            psumT[n][:, ci * BPC : (ci + 1) * BPC],
                            lhsT=evac[:, c * 128 : (c + 1) * 128],
                            rhs=i128[:],
                            start=True,
                            stop=True,
                        )

                def relu_group(n, hT_new=hT_new):
                    nc.vector.tensor_relu(
                        hT_new[:, 32 * n : 32 * n + 32], psumT[n][:, 0:32]
                    )

                # Emission is a topological schedule: MM starts are
                # pc-monotone on the PE, so every matmul is emitted at a
                # point where its inputs are already (or just-in-time)
                # available.  Rounds 0-2 of banks 0-2 only need relu groups
                # 0-2 of the previous microstep and stream through the
                # deferred bank-3 tail's latency window; the round-3 stops
                # follow the flushed tail; bank 3 and the in-microstep tails
                # fill the rest.
                if s == 0:
                    inj_mm(0)
                rounds(0, [0, 1, 2])
                if s == 0:
                    inj_mm(1)
                rounds(1, [0, 1, 2])
                for fn in (pending[:2] if pending else []):
                    fn()   # tmm_group(3)', relu_group(3)'
                if s == 0:
                    inj_mm(2)
                rounds(2, [0, 1, 2])
                rounds(0, [3])
                rounds(1, [3])
                rounds(2, [3])
                cast_bank(0)
                cast_bank(1)
                cast_bank(2)
                if s == 0:
                    inj_mm(3)
                rounds(3, [0, 1, 2])
                tmm_group(0)
                relu_group(0)
                for fn in (pending[2:] if pending else []):
                    fn()   # readout'
                rounds(3, [3])
                cast_bank(3)
                tmm_group(1)
                relu_group(1)
                tmm_group(2)
                relu_group(2)
                pending = [
                    lambda n=3, f=tmm_group: f(n),
                    lambda n=3, f=relu_group: f(n),
                ]

                if s == 3:
                    def readout(t=t, hT_new=hT_new):
                        # 16-chunk accumulation chain (pipelines at the
                        # 8-col issue rate), then ACT-copied out.
                        for c in range(NCHUNK):
                            nc.tensor.matmul(
                                psumT[3][0:2, PR : PR + BPC],
                                lhsT=wsel[:, c * 2 : (c + 1) * 2],
                                rhs=hT_new[:, c * BPC : (c + 1) * BPC],
                                start=(c == 0),
                                stop=(c == NCHUNK - 1),
                            )
                        nc.scalar.copy(
                            outst[:, t * BPC : (t + 1) * BPC],
                            psumT[3][0:2, PR : PR + BPC],
                        )

                    pending.append(readout)

                hT = hT_new

        for fn in pending:
            fn()
        nc.sync.dma_start(out_d[:], outst[:])
    nc.compile()
    return nc


def _prep_inputs(inputs, W_rec, W_in, b_in, W_out, sensory_indices, output_indices,
                 n_steps):
    inputs = np.asarray(inputs, np.float32)
    W_rec = np.asarray(W_rec, np.float32)
    W_in = np.asarray(W_in, np.float32)
    b_in = np.asarray(b_in, np.float32)
    W_out = np.asarray(W_out, np.float32)
    sens = np.asarray(sensory_indices).astype(np.int64)
    oidx = np.asarray(output_indices).astype(np.int64)

    wt = np.ascontiguousarray(W_rec.T).astype(np.float16)

    # scatter-expanded readout weights, feature-major by chunk
    wsel_full = np.zeros((2, N), np.float32)
    np.add.at(wsel_full, (slice(None), oidx), W_out)
    wsel = np.ascontiguousarray(
        wsel_full.reshape(2, NCHUNK, 128).transpose(2, 1, 0).reshape(128, 2 * NCHUNK)
    ).astype(np.float16)

    i128 = (np.arange(128)[:, None] % 32 == np.arange(BPC)[None, :]).astype(np.float16)

    # scatter-expanded injection weights: rows 0-3 = W_in.T, row 4 = b_in
    winj_full = np.zeros((8, N), np.float32)
    np.add.at(winj_full, (slice(None, 4), sens), W_in.T)
    np.add.at(winj_full[4], sens, b_in)
    winj = winj_full.astype(np.float16)

    # per-core x_t stationary blocks: [8, T*8], rows 0-3 = x_t^T, row 4 = 1
    ncin = inputs.shape[2]
    xt_cores = []
    for g in range(NCORES):
        a = inputs[g * BPC : (g + 1) * BPC, :n_steps, :]  # [8, T, 4]
        x = np.zeros((8, n_steps, BPC), np.float32)
        x[:ncin] = a.transpose(2, 1, 0)
        x[4] = 1.0
        xt_cores.append(np.ascontiguousarray(x.reshape(8, n_steps * BPC)).astype(np.float16))

    return wt, winj, xt_cores, wsel, i128


def _run(inputs, W_rec, W_in, b_in, W_out, b_out, sensory_indices, output_indices,
         K, n_steps=T, trace=False):
    from concourse.bass_utils import run_bass_kernel_spmd

    assert int(K) == 4
    wt, winj, xt_cores, wsel, i128 = _prep_inputs(
        inputs, W_rec, W_in, b_in, W_out, sensory_indices, output_indices, n_steps)

    if n_steps not in _CACHE:
        _CACHE[n_steps] = _build_nc(n_steps)
    nc = _CACHE[n_steps]

    in_maps = [
        {"wt": wt, "winj": winj, "xt": xt_cores[g], "wsel": wsel, "i128": i128}
        for g in range(NCORES)
    ]
    try:
        res = run_bass_kernel_spmd(nc, in_maps, list(range(NCORES)), trace=trace)
    except Exception:
        # transient device errors recover on re-execution
        res = run_bass_kernel_spmd(nc, in_maps, list(range(NCORES)), trace=trace)

    b_out = np.asarray(b_out, np.float32)
    outs = []
    for g in range(NCORES):
        r = np.asarray(res.results[g]["out"])  # [2, T*8]
        outs.append(r.reshape(2, n_steps, BPC).transpose(2, 1, 0))  # [8, T, 2]
    full = np.concatenate(outs, axis=0) + b_out  # [B, T, 2]
    return np.ascontiguousarray(full.astype(np.float32)), res


def kernel(**inputs):
    out, _ = _run(
        inputs["inputs"], inputs["W_rec"], inputs["W_in"], inputs["b_in"],
        inputs["W_out"], inputs["b_out"], inputs["sensory_indices"],
        inputs["output_indices"], inputs["K"],
    )
    return out
